# revision 61
# baseline (speedup 1.0000x reference)
"""Multi-head causal self-attention on 8 TRN2 NeuronCores (Bass/Tile).

Sharding: head + batch parallel. Core c handles batch b = c//4 and head
group g = c%4 (4 of 16 heads). Each core computes q/k/v projections for
its heads (K/V stay core-local), causal attention in a transposed
layout (scores^T: keys on partitions, queries on free dim), and a
partial o-projection against its 256 rows of Wo. The host sums the 4
per-batch partials (the tensor-parallel all-reduce) during unshard.

All matmuls run in bf16 with fp32 PSUM accumulation; softmax skips the
max-subtraction (scores are O(1) here: |s|/sqrt(dh) < ~3) and folds the
1/sqrt(dh) scale into the ACT exp. The softmax denominator rides along
in the attention-value matmul as an extra all-ones column of V.

Schedule: 4 "eras" (one per 512-token chunk), attends for query blocks
2t, 2t+1 with both head-pairs interleaved so the ACT-engine exp load is
spread evenly against PE work; era-t+1 projections and o-projections
ride as filler units popped between attend groups. O-projections are
fine-grained single-m-block units (~0.8us) so a pop never stalls the
exp stream. AV emission trails the scores/exp stream through a
cross-attend pending queue (batch drains; greedy on the final attend
with pops only in its early groups), with the AV PSUM accumulators
allocated lazily at first emission so single-buffered slot reuse stays
visible to the Tile scheduler. Normalization copies the denominator
rows out of PSUM ahead of the bulk staging so the reciprocal-broadcast
chain starts early; the final norm stages via the post-exp-idle ACT
engine and its o-projection drains casts/stores over the scalar,
vector, sync and gpsimd rings in parallel, with chunk-2's o-projection
held back to cover the final normalization chain. The diagonal
attention group skips the fully-masked half of its second key chunk;
all inputs arrive host-packed in SBUF tile layout so DMA descriptors
are full 2-4KB partition rows.
"""

import os
import sys
import types

import numpy as np
import ml_dtypes

BF16 = ml_dtypes.bfloat16

B = 2
S = 2048
D = 1024
H = 16
DH = 64
N_CORES = 8
HPC = 4  # heads per core
QB = 256  # query block
KC = 128  # key chunk

def _install_ntff_hook():
    """Best-effort: register the NTFF profile hook missing from this
    image's antenv, so BASS_TRACE=1 runs can report exec_time_ns."""
    if "antenv.axon_hooks" in sys.modules:
        return
    try:
        from trn_agent_boot.trn_boot import _ntff_profile_via_ctypes

        hook = _ntff_profile_via_ctypes("/opt/axon/libaxon_pjrt.so")
        mod = types.ModuleType("antenv.axon_hooks")
        mod.get_axon_ntff_profile_hook = lambda: hook
        mod.set_axon_ntff_profile_hook = lambda h: None
        sys.modules["antenv.axon_hooks"] = mod
    except Exception:
        pass


_BUILD_CACHE = {}


def _build(seq):
    """Build + compile the per-core SPMD program for sequence length seq."""
    if seq in _BUILD_CACHE:
        return _BUILD_CACHE[seq]

    import concourse.bass as bass  # noqa: F401
    import concourse.mybir as mybir
    import concourse.tile as tile
    from concourse import bacc

    f32 = mybir.dt.float32
    bf16 = mybir.dt.bfloat16
    Exp = mybir.ActivationFunctionType.Exp

    n_qb = seq // QB  # query blocks per head (8)
    n_t512 = seq // 512  # 512-token chunks (4)
    n_t128 = seq // KC  # 128-token chunks (16)
    CPC = HPC * DH  # columns per core (256)

    nc = bacc.Bacc("TRN2", target_bir_lowering=False, debug=False, num_devices=N_CORES)

    # inputs arrive pre-packed in tile layout so every DMA descriptor is a
    # full contiguous SBUF partition row (2-4KB), not a 0.5-1KB strided chunk
    xT_d = nc.dram_tensor(
        "xt", [seq // 512, 2, 128, 4, 512], bf16, kind="ExternalInput"
    ).ap()
    wq_d = nc.dram_tensor("wq", [2, 128, 4, CPC], bf16, kind="ExternalInput").ap()
    wk_d = nc.dram_tensor("wk", [2, 128, 4, CPC], bf16, kind="ExternalInput").ap()
    wv_d = nc.dram_tensor("wv", [2, 128, 4, CPC], bf16, kind="ExternalInput").ap()
    wo_d = nc.dram_tensor("wo", [128, 2, D], bf16, kind="ExternalInput").ap()
    mab_d = nc.dram_tensor("maskab", [KC, 512], bf16, kind="ExternalInput").ap()
    out_d = nc.dram_tensor("ot", [D, seq], bf16, kind="ExternalOutput").ap()

    with tile.TileContext(nc) as tc:
        with (
            tc.tile_pool(name="const", bufs=1) as const,
            tc.tile_pool(name="work", bufs=4) as work,
            tc.tile_pool(name="ps_sc", bufs=2, space="PSUM") as ps_sc,
            tc.tile_pool(name="ps_av", bufs=1, space="PSUM") as ps_av,
            tc.tile_pool(name="ps_pj", bufs=2, space="PSUM") as ps_pj,
        ):
            wq_r, wk_r, wv_r = wq_d, wk_d, wv_d

            # ---- input tiles; DMA issue order = first-need order, with
            # the first x / wq / wk chunks split per 128-row block so the
            # first projection matmuls gate on ~128KB, not megabytes ----
            wq_h = [const.tile([128, 4, CPC], bf16, name=f"wq{h}") for h in range(2)]
            wk_h = [const.tile([128, 4, CPC], bf16, name=f"wk{h}") for h in range(2)]
            wv_h = [const.tile([128, 4, CPC], bf16, name=f"wv{h}") for h in range(2)]
            xts = [
                [const.tile([128, 4, 512], bf16, name=f"xt{t}_{h}") for h in range(2)]
                for t in range(n_t512)
            ]

            def load(tl, src, ranges):
                for c0, c1 in ranges:
                    nc.sync.dma_start(tl[:, c0:c1, :], src[:, c0:c1, :])

            ONE = [(0, 1), (1, 2), (2, 3), (3, 4)]
            TWO = [(0, 2), (2, 4)]
            ALL = [(0, 4)]

            load(wq_h[0], wq_r[0], ONE)
            load(xts[0][0], xT_d[0, 0], ONE)
            load(wq_h[1], wq_r[1], TWO)
            load(xts[0][1], xT_d[0, 1], ONE)
            mab_sb = const.tile([KC, 512], bf16)
            nc.sync.dma_start(mab_sb[:], mab_d[:])
            load(wk_h[0], wk_r[0], TWO)
            load(wk_h[1], wk_r[1], TWO)
            load(wv_h[0], wv_r[0], TWO)
            load(wv_h[1], wv_r[1], TWO)
            for t in range(1, n_t512):
                rg = ONE if t == 1 else ALL
                load(xts[t][0], xT_d[t, 0], rg)
                load(xts[t][1], xT_d[t, 1], rg)
            wo_sb = const.tile([128, 2, D], bf16, name="wo_sb")
            nc.sync.dma_start(wo_sb[:], wo_d[:])

            # all-ones row at partition 64 (same partition as the AV tiles'
            # denominator row) for the final norm's PE-side broadcast
            ones64 = const.tile([65, 64], f32, name="ones64")
            nc.vector.memset(ones64[64:65, :], 1.0)

            # qTs[pair]: partitions = W cols [128*pair, 128*pair+128)
            # = heads (2*pair, 2*pair+1) x 64 dh.
            qTs = [const.tile([128, seq], bf16, name=f"qT{p}") for p in range(2)]
            kTs = [const.tile([128, seq], bf16, name=f"kT{p}") for p in range(2)]
            vs = [
                const.tile([128, HPC, DH + 1], bf16, name=f"v{t}")
                for t in range(n_t128)
            ]
            attns = [
                [const.tile([128, 512], bf16, name=f"at{p}_{t}") for t in range(n_t512)]
                for p in range(2)
            ]

            # ---- work units (projections / o-proj) ----
            def qk_unit(pair, t, which):
                w_h, dsts = ((wq_h, qTs) if which == "q" else (wk_h, kTs))
                ps = ps_pj.tile([128, 512], f32, tag="pj", name="pj")
                for kc in range(8):
                    nc.tensor.matmul(
                        ps[:],
                        lhsT=w_h[kc // 4][:, kc % 4, 128 * pair : 128 * pair + 128],
                        rhs=xts[t][kc // 4][:, kc % 4, :],
                        start=(kc == 0),
                        stop=(kc == 7),
                    )
                nc.vector.tensor_copy(dsts[pair][:, 512 * t : 512 * t + 512], ps[:])

            def v_unit(t):
                # vs[t][:, h, 0:64] = v values, [..., 64] = 1.0 (denom row)
                nc.vector.memset(vs[t][:, :, DH], 1.0)
                ps = ps_pj.tile([128, 512], f32, tag="pj", name="pv")
                for kc in range(8):
                    nc.tensor.matmul(
                        ps[:, :CPC],
                        lhsT=xts[t // 4][kc // 4][:, kc % 4,
                                                  KC * (t % 4) : KC * (t % 4) + KC],
                        rhs=wv_h[kc // 4][:, kc % 4, :],
                        start=(kc == 0),
                        stop=(kc == 7),
                    )
                nc.vector.tensor_copy(
                    vs[t][:, :, 0:DH],
                    ps[:, :CPC].rearrange("p (h d) -> p h d", h=HPC),
                )

            def o_block_t(t, m):
                # one 128-row output block of the full-512-query o-projection
                # for chunk t; self-contained so it can pop as a ~0.8us filler
                ps = ps_pj.tile([128, 512], f32, tag="pj", name="po")
                for pair in range(2):
                    nc.tensor.matmul(
                        ps[:],
                        lhsT=wo_sb[:, pair, 128 * m : 128 * m + 128],
                        rhs=attns[pair][t][:, :],
                        start=(pair == 0),
                        stop=(pair == 1),
                    )
                # 8-deep staging ring: with only 4, the cast for block k+4
                # waits on block k's multi-us store transfer to free the slot,
                # self-throttling o-proj bursts
                osb = work.tile([128, 512], bf16, tag="osb2", name="osb2",
                                bufs=8)
                nc.vector.tensor_copy(osb[:], ps[:])
                nc.sync.dma_start(
                    out_d[128 * m : 128 * m + 128, 512 * t : 512 * t + 512],
                    osb[:],
                )

            def o_block(qb, m):
                # one 128-row output block of the 256-query o-projection
                t, half = qb // 2, qb % 2
                endgame = qb == n_qb - 1
                aqs = slice(QB * half, QB * half + QB)
                ps = ps_pj.tile([128, 512], f32, tag="pj", name="po")
                for pair in range(2):
                    nc.tensor.matmul(
                        ps[:, 0:QB],
                        lhsT=wo_sb[:, pair, 128 * m : 128 * m + 128],
                        rhs=attns[pair][t][:, aqs],
                        start=(pair == 0),
                        stop=(pair == 1),
                    )
                osb = work.tile([128, QB], bf16, tag="osb", name="osb",
                                bufs=8)
                if endgame and m % 2 == 0:
                    # after the last exp the ACT engine is idle: split the
                    # final casts across scalar+vector and the stores across
                    # sync+gpsimd (half-blocks each) so no single ring or DMA
                    # engine serializes the drain
                    nc.scalar.copy(osb[:], ps[:, 0:QB])
                else:
                    nc.vector.tensor_copy(osb[:], ps[:, 0:QB])
                dst = out_d[128 * m : 128 * m + 128, QB * qb : QB * qb + QB]
                if endgame:
                    nc.sync.dma_start(dst[0:64, :], osb[0:64, :])
                    nc.gpsimd.dma_start(dst[64:128, :], osb[64:128, :])
                else:
                    nc.sync.dma_start(dst, osb[:])

            # ---- filler machinery: units pop between attend groups so the
            # PE never starves while ACT chews on exp ----
            units = []
            tail_units = []

            def pop_unit():
                if units:
                    units.pop(0)[1]()

            def drain_units(pred):
                i = 0
                while i < len(units):
                    if pred(units[i][0]):
                        _, fn = units.pop(i)
                        fn()
                    else:
                        i += 1

            # ---- attention ----
            # 256-query blocks, two key chunks per group. Scores run as
            # row-tiled 64-contraction pairs (both heads concurrent on the
            # two PE array halves); AV matmuls carry the softmax denominator
            # in the ones-column of V. The diagonal group trims the fully
            # masked half of its second key chunk.
            SKEW = 2
            pending = []

            def emit_av(item):
                exp_sb, g, hold, nchunks, pair, qb, diag = item
                if hold["avs"] is None:
                    # lazy PSUM allocation: all earlier attends' AV writes and
                    # norm reads are already emitted (FIFO), so the WAR on the
                    # single-buffered slots is visible to the Tile scheduler
                    hold["avs"] = [
                        ps_av.tile([DH + 1, QB], f32, tag=f"av{s}",
                                   name=f"av{s}", bufs=1)
                        for s in range(2)
                    ]
                avs = hold["avs"]
                for sub in range(2):
                    h = 2 * pair + sub
                    for j in range(2):
                        c = 2 * g + j
                        trim = diag and j == 1
                        n = KC if trim else QB
                        coff = KC if trim else 0
                        nc.tensor.matmul(
                            avs[sub][:, coff : coff + n],
                            lhsT=vs[c][:, h, :],
                            rhs=exp_sb[:, 512 * sub + QB * j :
                                       512 * sub + QB * j + n],
                            start=(c == 0),
                            stop=(c == nchunks - 1),
                            skip_group_check=diag,
                        )
                if diag:
                    norm(avs, pair, qb)

            def norm(avs, pair, qb):
                t, half = qb // 2, qb % 2
                at = attns[pair][t]
                aqs = slice(QB * half, QB * half + QB)
                last = pair == 1 and qb == n_qb - 1
                # Copy raw AV (values + ones-row sums) out of PSUM first so
                # the PSUM slot frees immediately; normalize from SBUF.
                # denominator rows leave PSUM first (tiny copies) so the
                # reciprocal-broadcast chain starts before the bulk staging;
                # the final norm's copies ride the then-idle ACT engine
                cp = nc.scalar.copy if last else nc.vector.tensor_copy
                den = work.tile([65, 2 * QB], f32, tag="den", name="den")
                for s in range(2):
                    cp(den[64:65, QB * s : QB * s + QB], avs[s][64:65, :])
                rb0 = work.tile([1, 2 * QB], f32, tag="rb0", name="rb0")
                (nc.gpsimd if last else nc.sync).dma_start(rb0[:],
                                                           den[64:65, :])
                avu = work.tile([65, 2 * QB], bf16, tag="avu", name="avu")
                for s in range(2):
                    cp(avu[:, QB * s : QB * s + QB], avs[s][:, :])
                dbc = work.tile([64, 2 * QB], f32, tag="dbc", name="dbc")
                nc.gpsimd.partition_broadcast(dbc[:], rb0[:])
                bcast = work.tile([64, 2 * QB], f32, tag="bcast",
                                  name="bcast")
                nc.vector.reciprocal_approx_fast(out=bcast[:], in_=dbc[:])
                nc.vector.tensor_mul(at[0:64, aqs], avu[0:64, 0:QB],
                                     bcast[:, 0:QB])
                tmp = work.tile([64, QB], bf16, tag="tmp", name="tmp")
                nc.vector.tensor_mul(tmp[:], avu[0:64, QB:], bcast[:, QB:])
                (nc.scalar if last else nc.sync).dma_start(
                    at[64:128, aqs], tmp[:]
                )
                if pair == 1 and qb >= 2 * n_t512 - 2:
                    # last era's own o-projections go out as soon as ready
                    for m in range(8):
                        units.append((("om", qb, m),
                                      lambda q=qb, mm=m: o_block(q, mm)))

            def attend(pair, qb, greedy=False):
                nchunks = 2 * qb + 2
                ngroups = nchunks // 2
                hold = {"avs": None}
                for g in range(ngroups):
                    diag = g == ngroups - 1
                    sc = ps_sc.tile([128, 1024], f32, tag="sc", name="sc")
                    for j in range(2):
                        trim = diag and j == 1
                        n = KC if trim else QB
                        qoff = QB * qb + (KC if trim else 0)
                        for sub in range(2):
                            c = 2 * g + j
                            p0 = 64 * sub
                            nc.tensor.matmul(
                                sc[:, 512 * sub + QB * j :
                                   512 * sub + QB * j + n],
                                lhsT=kTs[pair][p0 : p0 + 64,
                                               KC * c : KC * c + KC],
                                rhs=qTs[pair][p0 : p0 + 64, qoff : qoff + n],
                                start=True,
                                stop=True,
                            )
                    exp_sb = work.tile([128, 1024], bf16, tag="exp", name="exp",
                                       bufs=10)
                    if diag:
                        # both heads' diagonal regions in ONE strided exp op
                        # and two broadcast mask-muls, halving the per-group
                        # ACT/DVE instruction overhead in the diag groups
                        e2 = exp_sb[:].rearrange("p (s c) -> p s c", s=2)
                        s2 = sc[:].rearrange("p (s c) -> p s c", s=2)
                        nc.scalar.activation(
                            e2[:, :, 0 : QB + KC],
                            s2[:, :, 0 : QB + KC],
                            Exp,
                            scale=0.125,
                        )
                        nc.vector.tensor_mul(
                            e2[:, :, 0:QB],
                            e2[:, :, 0:QB],
                            mab_sb[:, 0:QB].unsqueeze(1).to_broadcast(
                                [KC, 2, QB]),
                        )
                        nc.vector.tensor_mul(
                            e2[:, :, QB : QB + KC],
                            e2[:, :, QB : QB + KC],
                            mab_sb[:, 0:KC].unsqueeze(1).to_broadcast(
                                [KC, 2, KC]),
                        )
                    else:
                        nc.scalar.activation(exp_sb[:], sc[:], Exp, scale=0.125)
                    pending.append((exp_sb, g, hold, nchunks, pair, qb, diag))
                    if greedy:
                        # ACT-paced endgame: drain AVs eagerly. One fine
                        # filler block pops per early group; the last groups
                        # stay clean so the final norm chain isn't queued
                        # behind filler casts in the strict FIFO.
                        while len(pending) > 1:
                            emit_av(pending.pop(0))
                        if g % 2 == 0 and g < ngroups - 2 and len(units) > 4:
                            # keep ~4 blocks in reserve: together with the
                            # held-back chunk-2 o-proj they cover the final
                            # norm chain's PE window after the flush
                            pop_unit()
                    elif len(pending) >= SKEW + 4:
                        for _ in range(4):
                            emit_av(pending.pop(0))
                        for _ in range(3 if qb >= 6 else 2):
                            pop_unit()
                if not greedy:
                    pop_unit()
                    if qb >= 6:
                        pop_unit()

            # ---- era schedule ----
            # era 0 prologue: projections for t=0 run up front (nothing to
            # overlap them with yet)
            # v-units run AFTER the first attends: AV emission trails through
            # the pending queue, so v isn't needed until several groups
            # later, and this gets the first exp ~1.7us earlier
            qk_unit(0, 0, "q")
            qk_unit(0, 0, "k")
            attend(0, 0)
            qk_unit(1, 0, "q")
            qk_unit(1, 0, "k")
            attend(1, 0)
            v_unit(0)
            v_unit(1)
            v_unit(2)
            v_unit(3)

            for t in range(n_t512):
                if t + 1 < n_t512:
                    for pair in range(2):
                        for w in ("q", "k"):
                            units.append(
                                (("qk", pair, t + 1, w),
                                 lambda p=pair, tt=t + 1, ww=w: qk_unit(p, tt, ww))
                            )
                    for c in range(4 * t + 4, 4 * t + 8):
                        units.append((("v", c), lambda cc=c: v_unit(cc)))
                if t == n_t512 - 1:
                    # reserved o-projections: the last era has no projection
                    # fillers, so it absorbs all earlier chunks' o-proj as
                    # fine-grained m-block units. Half of chunk t-2 is held
                    # back to cover the final normalization chain; the rest
                    # pops between attend groups.
                    for tt in range(n_t512 - 2):
                        for m in range(8):
                            units.append((("otm", tt, m),
                                          lambda x=tt, mm=m: o_block_t(x, mm)))
                    tt = n_t512 - 2
                    for m in range(8):
                        tail_units.append(lambda x=tt, mm=m: o_block_t(x, mm))

                def need(key, tt=t):
                    return (key[0] == "qk" and key[2] == tt) or (
                        key[0] == "v" and key[1] <= 4 * tt + 3
                    )

                drain_units(need)
                if t > 0:
                    attend(0, 2 * t)
                    attend(1, 2 * t)
                attend(0, 2 * t + 1)
                attend(1, 2 * t + 1, greedy=(t == n_t512 - 1))

            while pending:
                emit_av(pending.pop(0))
            for fn in tail_units:
                fn()
            while units:
                units.pop(0)[1]()

    nc.compile()
    _BUILD_CACHE[seq] = nc
    return nc


def _masks():
    """Lower-triangle keep mask over a 512-query block: keep key j for
    query offset i when j <= i. Diagonal chunk d uses columns
    [0, 512-128d) against its valid query suffix."""
    j = np.arange(KC)[:, None]
    i = np.arange(512)[None, :]
    return (j <= i).astype(BF16)


def _run(x, Wq, Wk, Wv, Wo, seq, trace=False):
    from concourse import bass_utils

    if trace or os.environ.get("BASS_TRACE"):
        _install_ntff_hook()
    nc = _build(seq)

    maskab = _masks()

    def pack_x(xb):
        # [S, D] -> tile layout [t, h, p, c, s] with D-row = h*512 + c*128 + p
        xT = xb.T.astype(BF16)  # [D, S]
        arr = xT.reshape(2, 4, 128, seq // 512, 512)  # (h, c, p, t, s)
        return np.ascontiguousarray(arr.transpose(3, 0, 2, 1, 4))

    def pack_w(w):
        # [D, cols-per-core] -> [h, p, c, m]
        arr = w.astype(BF16).reshape(2, 4, 128, HPC * DH)  # (h, c, p, m)
        return np.ascontiguousarray(arr.transpose(0, 2, 1, 3))

    xP = [pack_x(x[b]) for b in range(B)]

    in_maps = []
    for c in range(N_CORES):
        b, g = c // HPC, c % HPC
        cols = slice(HPC * DH * g, HPC * DH * (g + 1))
        wo_g = Wo[cols, :].astype(BF16)  # [CPC, D]
        wo_p = np.ascontiguousarray(
            wo_g.reshape(2, 128, D).transpose(1, 0, 2)  # [p, c, m]
        )
        in_maps.append(
            {
                "xt": xP[b],
                "wq": pack_w(Wq[:, cols]),
                "wk": pack_w(Wk[:, cols]),
                "wv": pack_w(Wv[:, cols]),
                "wo": wo_p,
                "maskab": maskab,
            }
        )

    res = bass_utils.run_bass_kernel_spmd(
        nc, in_maps, core_ids=list(range(N_CORES)), trace=trace
    )
    if res.exec_time_ns is not None:
        print(f"HW exec time: {res.exec_time_ns} ns")

    out = np.zeros((B, seq, D), dtype=np.float32)
    for c in range(N_CORES):
        b = c // HPC
        out[b] += res.results[c]["ot"].T.astype(np.float32)
    return out


def kernel(x, Wq, Wk, Wv, Wo):
    x = np.asarray(x, dtype=np.float32)
    return _run(
        x,
        np.asarray(Wq, np.float32),
        np.asarray(Wk, np.float32),
        np.asarray(Wv, np.float32),
        np.asarray(Wo, np.float32),
        seq=x.shape[1],
        trace=bool(os.environ.get("BASS_TRACE")),
    )



# revision 63
# speedup vs baseline: 1.1780x; 1.1780x over previous
"""Multi-head causal self-attention on 8 TRN2 NeuronCores (Bass/Tile).

Sharding: head + batch parallel. Core c handles batch b = c//4 and head
group g = c%4 (4 of 16 heads). Each core computes q/k/v projections for
its heads (K/V stay core-local), causal attention in a transposed
layout (scores^T: keys on partitions, queries on free dim), and a
partial o-projection against its 256 rows of Wo. The host sums the 4
per-batch partials (the tensor-parallel all-reduce) during unshard.

All matmuls run in bf16 with fp32 PSUM accumulation; softmax skips the
max-subtraction (scores are O(1) here: |s|/sqrt(dh) < ~3) and folds the
1/sqrt(dh) scale into the ACT exp. The softmax denominator rides along
in the attention-value matmul as an extra all-ones column of V.

Schedule: 4 "eras" (one per 512-token chunk), attends for query blocks
2t, 2t+1 with both head-pairs interleaved so the ACT-engine exp load is
spread evenly against PE work; era-t+1 projections and o-projections
ride as filler units popped between attend groups. O-projections are
fine-grained single-m-block units (~0.8us) so a pop never stalls the
exp stream. AV emission trails the scores/exp stream through a
cross-attend pending queue (batch drains; greedy on the final attend
with pops only in its early groups), with the AV PSUM accumulators
allocated lazily at first emission so single-buffered slot reuse stays
visible to the Tile scheduler. Normalization copies the denominator
rows out of PSUM ahead of the bulk staging so the reciprocal-broadcast
chain starts early; the final norm stages via the post-exp-idle ACT
engine and its o-projection drains casts/stores over the scalar,
vector, sync and gpsimd rings in parallel, with chunk-2's o-projection
held back to cover the final normalization chain. The diagonal
attention group skips the fully-masked half of its second key chunk;
all inputs arrive host-packed in SBUF tile layout so DMA descriptors
are full 2-4KB partition rows.
"""

import os
import sys
import types

import numpy as np
import ml_dtypes

BF16 = ml_dtypes.bfloat16

B = 2
S = 2048
D = 1024
H = 16
DH = 64
N_CORES = 8
HPC = 4  # heads per core
QB = 256  # query block
KC = 128  # key chunk

def _install_ntff_hook():
    """Best-effort: register the NTFF profile hook missing from this
    image's antenv, so BASS_TRACE=1 runs can report exec_time_ns."""
    if "antenv.axon_hooks" in sys.modules:
        return
    try:
        from trn_agent_boot.trn_boot import _ntff_profile_via_ctypes

        hook = _ntff_profile_via_ctypes("/opt/axon/libaxon_pjrt.so")
        mod = types.ModuleType("antenv.axon_hooks")
        mod.get_axon_ntff_profile_hook = lambda: hook
        mod.set_axon_ntff_profile_hook = lambda h: None
        sys.modules["antenv.axon_hooks"] = mod
    except Exception:
        pass


_BUILD_CACHE = {}


def _build(seq):
    """Build + compile the per-core SPMD program for sequence length seq."""
    if seq in _BUILD_CACHE:
        return _BUILD_CACHE[seq]

    import concourse.bass as bass  # noqa: F401
    import concourse.mybir as mybir
    import concourse.tile as tile
    from concourse import bacc

    f32 = mybir.dt.float32
    bf16 = mybir.dt.bfloat16
    Exp = mybir.ActivationFunctionType.Exp

    n_qb = seq // QB  # query blocks per head (8)
    n_t512 = seq // 512  # 512-token chunks (4)
    n_t128 = seq // KC  # 128-token chunks (16)
    CPC = HPC * DH  # columns per core (256)

    nc = bacc.Bacc("TRN2", target_bir_lowering=False, debug=False, num_devices=N_CORES)

    # inputs arrive pre-packed in tile layout so every DMA descriptor is a
    # full contiguous SBUF partition row (2-4KB), not a 0.5-1KB strided chunk
    xT_d = nc.dram_tensor(
        "xt", [seq // 512, 2, 128, 4, 512], bf16, kind="ExternalInput"
    ).ap()
    wq_d = nc.dram_tensor("wq", [2, 128, 4, CPC], bf16, kind="ExternalInput").ap()
    wk_d = nc.dram_tensor("wk", [2, 128, 4, CPC], bf16, kind="ExternalInput").ap()
    wv_d = nc.dram_tensor("wv", [2, 128, 4, CPC], bf16, kind="ExternalInput").ap()
    wo_d = nc.dram_tensor("wo", [128, 2, D], bf16, kind="ExternalInput").ap()
    mab_d = nc.dram_tensor("maskab", [KC, 512], bf16, kind="ExternalInput").ap()
    out_d = nc.dram_tensor("ot", [D, seq], bf16, kind="ExternalOutput").ap()

    with tile.TileContext(nc) as tc:
        with (
            tc.tile_pool(name="const", bufs=1) as const,
            tc.tile_pool(name="work", bufs=4) as work,
            tc.tile_pool(name="ps_sc", bufs=2, space="PSUM") as ps_sc,
            tc.tile_pool(name="ps_av", bufs=1, space="PSUM") as ps_av,
            tc.tile_pool(name="ps_pj", bufs=2, space="PSUM") as ps_pj,
        ):
            wq_r, wk_r, wv_r = wq_d, wk_d, wv_d

            # ---- input tiles; DMA issue order = first-need order, with
            # the first x / wq / wk chunks split per 128-row block so the
            # first projection matmuls gate on ~128KB, not megabytes ----
            wq_h = [const.tile([128, 4, CPC], bf16, name=f"wq{h}") for h in range(2)]
            wk_h = [const.tile([128, 4, CPC], bf16, name=f"wk{h}") for h in range(2)]
            wv_h = [const.tile([128, 4, CPC], bf16, name=f"wv{h}") for h in range(2)]
            xts = [
                [const.tile([128, 4, 512], bf16, name=f"xt{t}_{h}") for h in range(2)]
                for t in range(n_t512)
            ]

            def load(tl, src, ranges):
                for c0, c1 in ranges:
                    nc.sync.dma_start(tl[:, c0:c1, :], src[:, c0:c1, :])

            ONE = [(0, 1), (1, 2), (2, 3), (3, 4)]
            TWO = [(0, 2), (2, 4)]
            ALL = [(0, 4)]

            # interleave wq0 / x-t0 chunks: the kc-th projection matmul needs
            # one chunk of EACH, so alternating completions unblock the
            # accumulation chain earliest
            for c in range(4):
                load(wq_h[0], wq_r[0], [(c, c + 1)])
                load(xts[0][0], xT_d[0, 0], [(c, c + 1)])
            load(wq_h[1], wq_r[1], TWO)
            load(xts[0][1], xT_d[0, 1], ONE)
            mab_sb = const.tile([KC, 512], bf16)
            nc.sync.dma_start(mab_sb[:], mab_d[:])
            load(wk_h[0], wk_r[0], TWO)
            load(wk_h[1], wk_r[1], TWO)
            load(wv_h[0], wv_r[0], TWO)
            load(wv_h[1], wv_r[1], TWO)
            for t in range(1, n_t512):
                rg = ONE if t == 1 else ALL
                load(xts[t][0], xT_d[t, 0], rg)
                load(xts[t][1], xT_d[t, 1], rg)
            wo_sb = const.tile([128, 2, D], bf16, name="wo_sb")
            nc.sync.dma_start(wo_sb[:], wo_d[:])

            # all-ones row at partition 64 (same partition as the AV tiles'
            # denominator row) for the final norm's PE-side broadcast
            ones64 = const.tile([65, 64], f32, name="ones64")
            nc.vector.memset(ones64[64:65, :], 1.0)

            # qTs[pair]: partitions = W cols [128*pair, 128*pair+128)
            # = heads (2*pair, 2*pair+1) x 64 dh.
            qTs = [const.tile([128, seq], bf16, name=f"qT{p}") for p in range(2)]
            kTs = [const.tile([128, seq], bf16, name=f"kT{p}") for p in range(2)]
            vs = [
                const.tile([128, HPC, DH + 1], bf16, name=f"v{t}")
                for t in range(n_t128)
            ]
            attns = [
                [const.tile([128, 512], bf16, name=f"at{p}_{t}") for t in range(n_t512)]
                for p in range(2)
            ]

            # ---- work units (projections / o-proj) ----
            def qk_unit(pair, t, which):
                w_h, dsts = ((wq_h, qTs) if which == "q" else (wk_h, kTs))
                ps = ps_pj.tile([128, 512], f32, tag="pj", name="pj")
                for kc in range(8):
                    nc.tensor.matmul(
                        ps[:],
                        lhsT=w_h[kc // 4][:, kc % 4, 128 * pair : 128 * pair + 128],
                        rhs=xts[t][kc // 4][:, kc % 4, :],
                        start=(kc == 0),
                        stop=(kc == 7),
                    )
                nc.vector.tensor_copy(dsts[pair][:, 512 * t : 512 * t + 512], ps[:])

            def v_unit(t):
                # vs[t][:, h, 0:64] = v values, [..., 64] = 1.0 (denom row)
                nc.vector.memset(vs[t][:, :, DH], 1.0)
                ps = ps_pj.tile([128, 512], f32, tag="pj", name="pv")
                for kc in range(8):
                    nc.tensor.matmul(
                        ps[:, :CPC],
                        lhsT=xts[t // 4][kc // 4][:, kc % 4,
                                                  KC * (t % 4) : KC * (t % 4) + KC],
                        rhs=wv_h[kc // 4][:, kc % 4, :],
                        start=(kc == 0),
                        stop=(kc == 7),
                    )
                nc.vector.tensor_copy(
                    vs[t][:, :, 0:DH],
                    ps[:, :CPC].rearrange("p (h d) -> p h d", h=HPC),
                )

            def o_block_t(t, m):
                # one 128-row output block of the full-512-query o-projection
                # for chunk t; self-contained so it can pop as a ~0.8us filler
                ps = ps_pj.tile([128, 512], f32, tag="pj", name="po")
                for pair in range(2):
                    nc.tensor.matmul(
                        ps[:],
                        lhsT=wo_sb[:, pair, 128 * m : 128 * m + 128],
                        rhs=attns[pair][t][:, :],
                        start=(pair == 0),
                        stop=(pair == 1),
                    )
                # 8-deep staging ring: with only 4, the cast for block k+4
                # waits on block k's multi-us store transfer to free the slot,
                # self-throttling o-proj bursts
                osb = work.tile([128, 512], bf16, tag="osb2", name="osb2",
                                bufs=8)
                nc.vector.tensor_copy(osb[:], ps[:])
                nc.sync.dma_start(
                    out_d[128 * m : 128 * m + 128, 512 * t : 512 * t + 512],
                    osb[:],
                )

            def o_block(qb, m):
                # one 128-row output block of the 256-query o-projection
                t, half = qb // 2, qb % 2
                endgame = qb == n_qb - 1
                aqs = slice(QB * half, QB * half + QB)
                ps = ps_pj.tile([128, 512], f32, tag="pj", name="po")
                for pair in range(2):
                    nc.tensor.matmul(
                        ps[:, 0:QB],
                        lhsT=wo_sb[:, pair, 128 * m : 128 * m + 128],
                        rhs=attns[pair][t][:, aqs],
                        start=(pair == 0),
                        stop=(pair == 1),
                    )
                osb = work.tile([128, QB], bf16, tag="osb", name="osb",
                                bufs=8)
                if endgame and m % 2 == 0:
                    # after the last exp the ACT engine is idle: split the
                    # final casts across scalar+vector and the stores across
                    # sync+gpsimd (half-blocks each) so no single ring or DMA
                    # engine serializes the drain
                    nc.scalar.copy(osb[:], ps[:, 0:QB])
                else:
                    nc.vector.tensor_copy(osb[:], ps[:, 0:QB])
                dst = out_d[128 * m : 128 * m + 128, QB * qb : QB * qb + QB]
                if endgame:
                    nc.sync.dma_start(dst[0:64, :], osb[0:64, :])
                    nc.gpsimd.dma_start(dst[64:128, :], osb[64:128, :])
                else:
                    nc.sync.dma_start(dst, osb[:])

            # ---- filler machinery: units pop between attend groups so the
            # PE never starves while ACT chews on exp ----
            units = []
            tail_units = []

            def pop_unit():
                if units:
                    units.pop(0)[1]()

            def drain_units(pred):
                i = 0
                while i < len(units):
                    if pred(units[i][0]):
                        _, fn = units.pop(i)
                        fn()
                    else:
                        i += 1

            # ---- attention ----
            # 256-query blocks, two key chunks per group. Scores run as
            # row-tiled 64-contraction pairs (both heads concurrent on the
            # two PE array halves); AV matmuls carry the softmax denominator
            # in the ones-column of V. The diagonal group trims the fully
            # masked half of its second key chunk.
            SKEW = 2
            pending = []

            def emit_av(item):
                exp_sb, g, hold, nchunks, pair, qb, diag = item
                if hold["avs"] is None:
                    # lazy PSUM allocation: all earlier attends' AV writes and
                    # norm reads are already emitted (FIFO), so the WAR on the
                    # single-buffered slots is visible to the Tile scheduler
                    hold["avs"] = [
                        ps_av.tile([DH + 1, QB], f32, tag=f"av{s}",
                                   name=f"av{s}", bufs=1)
                        for s in range(2)
                    ]
                avs = hold["avs"]
                for sub in range(2):
                    h = 2 * pair + sub
                    for j in range(2):
                        c = 2 * g + j
                        trim = diag and j == 1
                        n = KC if trim else QB
                        coff = KC if trim else 0
                        nc.tensor.matmul(
                            avs[sub][:, coff : coff + n],
                            lhsT=vs[c][:, h, :],
                            rhs=exp_sb[:, 512 * sub + QB * j :
                                       512 * sub + QB * j + n],
                            start=(c == 0),
                            stop=(c == nchunks - 1),
                            skip_group_check=diag,
                        )
                if diag:
                    norm(avs, pair, qb)

            def norm(avs, pair, qb):
                t, half = qb // 2, qb % 2
                at = attns[pair][t]
                aqs = slice(QB * half, QB * half + QB)
                last = pair == 1 and qb == n_qb - 1
                # Copy raw AV (values + ones-row sums) out of PSUM first so
                # the PSUM slot frees immediately; normalize from SBUF.
                # denominator rows leave PSUM first (tiny copies) so the
                # reciprocal-broadcast chain starts before the bulk staging;
                # the final norm's copies ride the then-idle ACT engine
                cp = nc.scalar.copy if last else nc.vector.tensor_copy
                den = work.tile([65, 2 * QB], f32, tag="den", name="den")
                for s in range(2):
                    cp(den[64:65, QB * s : QB * s + QB], avs[s][64:65, :])
                rb0 = work.tile([1, 2 * QB], f32, tag="rb0", name="rb0")
                (nc.gpsimd if last else nc.sync).dma_start(rb0[:],
                                                           den[64:65, :])
                avu = work.tile([65, 2 * QB], bf16, tag="avu", name="avu")
                for s in range(2):
                    cp(avu[:, QB * s : QB * s + QB], avs[s][:, :])
                dbc = work.tile([64, 2 * QB], f32, tag="dbc", name="dbc")
                nc.gpsimd.partition_broadcast(dbc[:], rb0[:])
                bcast = work.tile([64, 2 * QB], f32, tag="bcast",
                                  name="bcast")
                nc.vector.reciprocal_approx_fast(out=bcast[:], in_=dbc[:])
                nc.vector.tensor_mul(at[0:64, aqs], avu[0:64, 0:QB],
                                     bcast[:, 0:QB])
                tmp = work.tile([64, QB], bf16, tag="tmp", name="tmp")
                nc.vector.tensor_mul(tmp[:], avu[0:64, QB:], bcast[:, QB:])
                (nc.scalar if last else nc.sync).dma_start(
                    at[64:128, aqs], tmp[:]
                )
                if pair == 1 and qb >= 2 * n_t512 - 2:
                    # last era's own o-projections go out as soon as ready
                    for m in range(8):
                        units.append((("om", qb, m),
                                      lambda q=qb, mm=m: o_block(q, mm)))

            def attend(pair, qb, greedy=False):
                nchunks = 2 * qb + 2
                ngroups = nchunks // 2
                hold = {"avs": None}
                for g in range(ngroups):
                    diag = g == ngroups - 1
                    sc = ps_sc.tile([128, 1024], f32, tag="sc", name="sc")
                    for j in range(2):
                        trim = diag and j == 1
                        n = KC if trim else QB
                        qoff = QB * qb + (KC if trim else 0)
                        for sub in range(2):
                            c = 2 * g + j
                            p0 = 64 * sub
                            nc.tensor.matmul(
                                sc[:, 512 * sub + QB * j :
                                   512 * sub + QB * j + n],
                                lhsT=kTs[pair][p0 : p0 + 64,
                                               KC * c : KC * c + KC],
                                rhs=qTs[pair][p0 : p0 + 64, qoff : qoff + n],
                                start=True,
                                stop=True,
                            )
                    exp_sb = work.tile([128, 1024], bf16, tag="exp", name="exp",
                                       bufs=12)
                    if diag:
                        # both heads' diagonal regions in ONE strided exp op
                        # and two broadcast mask-muls, halving the per-group
                        # ACT/DVE instruction overhead in the diag groups
                        e2 = exp_sb[:].rearrange("p (s c) -> p s c", s=2)
                        s2 = sc[:].rearrange("p (s c) -> p s c", s=2)
                        nc.scalar.activation(
                            e2[:, :, 0 : QB + KC],
                            s2[:, :, 0 : QB + KC],
                            Exp,
                            scale=0.125,
                        )
                        nc.vector.tensor_mul(
                            e2[:, :, 0:QB],
                            e2[:, :, 0:QB],
                            mab_sb[:, 0:QB].unsqueeze(1).to_broadcast(
                                [KC, 2, QB]),
                        )
                        nc.vector.tensor_mul(
                            e2[:, :, QB : QB + KC],
                            e2[:, :, QB : QB + KC],
                            mab_sb[:, 0:KC].unsqueeze(1).to_broadcast(
                                [KC, 2, KC]),
                        )
                    else:
                        nc.scalar.activation(exp_sb[:], sc[:], Exp, scale=0.125)
                    pending.append((exp_sb, g, hold, nchunks, pair, qb, diag))
                    if greedy:
                        # ACT-paced endgame: drain AVs eagerly. One fine
                        # filler block pops per early group; the last groups
                        # stay clean so the final norm chain isn't queued
                        # behind filler casts in the strict FIFO.
                        while len(pending) > 1:
                            emit_av(pending.pop(0))
                        if g % 2 == 0 and g < ngroups - 2 and len(units) > 4:
                            # keep ~4 blocks in reserve: together with the
                            # held-back chunk-2 o-proj they cover the final
                            # norm chain's PE window after the flush
                            pop_unit()
                    elif len(pending) >= SKEW + 4:
                        for _ in range(4):
                            emit_av(pending.pop(0))
                        for _ in range(3 if qb >= 6 else 2):
                            pop_unit()
                if not greedy:
                    pop_unit()
                    if qb >= 6:
                        pop_unit()

            # ---- era schedule ----
            # era 0 prologue: projections for t=0 run up front (nothing to
            # overlap them with yet)
            # v-units run AFTER the first attends: AV emission trails through
            # the pending queue, so v isn't needed until several groups
            # later, and this gets the first exp ~1.7us earlier
            qk_unit(0, 0, "q")
            qk_unit(0, 0, "k")
            attend(0, 0)
            qk_unit(1, 0, "q")
            qk_unit(1, 0, "k")
            attend(1, 0)
            v_unit(0)
            v_unit(1)
            v_unit(2)
            v_unit(3)

            for t in range(n_t512):
                if t + 1 < n_t512:
                    for pair in range(2):
                        for w in ("q", "k"):
                            units.append(
                                (("qk", pair, t + 1, w),
                                 lambda p=pair, tt=t + 1, ww=w: qk_unit(p, tt, ww))
                            )
                    for c in range(4 * t + 4, 4 * t + 8):
                        units.append((("v", c), lambda cc=c: v_unit(cc)))
                if t == n_t512 - 1:
                    # reserved o-projections: the last era has no projection
                    # fillers, so it absorbs all earlier chunks' o-proj as
                    # fine-grained m-block units. Half of chunk t-2 is held
                    # back to cover the final normalization chain; the rest
                    # pops between attend groups.
                    for tt in range(n_t512 - 2):
                        for m in range(8):
                            units.append((("otm", tt, m),
                                          lambda x=tt, mm=m: o_block_t(x, mm)))
                    tt = n_t512 - 2
                    for m in range(8):
                        tail_units.append(lambda x=tt, mm=m: o_block_t(x, mm))

                def need(key, tt=t):
                    return (key[0] == "qk" and key[2] == tt) or (
                        key[0] == "v" and key[1] <= 4 * tt + 3
                    )

                drain_units(need)
                if t > 0:
                    attend(0, 2 * t)
                    attend(1, 2 * t)
                attend(0, 2 * t + 1)
                attend(1, 2 * t + 1, greedy=(t == n_t512 - 1))

            while pending:
                emit_av(pending.pop(0))
            for fn in tail_units:
                fn()
            while units:
                units.pop(0)[1]()

    nc.compile()
    _BUILD_CACHE[seq] = nc
    return nc


def _masks():
    """Lower-triangle keep mask over a 512-query block: keep key j for
    query offset i when j <= i. Diagonal chunk d uses columns
    [0, 512-128d) against its valid query suffix."""
    j = np.arange(KC)[:, None]
    i = np.arange(512)[None, :]
    return (j <= i).astype(BF16)


def _run(x, Wq, Wk, Wv, Wo, seq, trace=False):
    from concourse import bass_utils

    if trace or os.environ.get("BASS_TRACE"):
        _install_ntff_hook()
    nc = _build(seq)

    maskab = _masks()

    def pack_x(xb):
        # [S, D] -> tile layout [t, h, p, c, s] with D-row = h*512 + c*128 + p
        xT = xb.T.astype(BF16)  # [D, S]
        arr = xT.reshape(2, 4, 128, seq // 512, 512)  # (h, c, p, t, s)
        return np.ascontiguousarray(arr.transpose(3, 0, 2, 1, 4))

    def pack_w(w):
        # [D, cols-per-core] -> [h, p, c, m]
        arr = w.astype(BF16).reshape(2, 4, 128, HPC * DH)  # (h, c, p, m)
        return np.ascontiguousarray(arr.transpose(0, 2, 1, 3))

    xP = [pack_x(x[b]) for b in range(B)]

    in_maps = []
    for c in range(N_CORES):
        b, g = c // HPC, c % HPC
        cols = slice(HPC * DH * g, HPC * DH * (g + 1))
        wo_g = Wo[cols, :].astype(BF16)  # [CPC, D]
        wo_p = np.ascontiguousarray(
            wo_g.reshape(2, 128, D).transpose(1, 0, 2)  # [p, c, m]
        )
        in_maps.append(
            {
                "xt": xP[b],
                "wq": pack_w(Wq[:, cols]),
                "wk": pack_w(Wk[:, cols]),
                "wv": pack_w(Wv[:, cols]),
                "wo": wo_p,
                "maskab": maskab,
            }
        )

    res = bass_utils.run_bass_kernel_spmd(
        nc, in_maps, core_ids=list(range(N_CORES)), trace=trace
    )
    if res.exec_time_ns is not None:
        print(f"HW exec time: {res.exec_time_ns} ns")

    out = np.zeros((B, seq, D), dtype=np.float32)
    for c in range(N_CORES):
        b = c // HPC
        out[b] += res.results[c]["ot"].T.astype(np.float32)
    return out


def kernel(x, Wq, Wk, Wv, Wo):
    x = np.asarray(x, dtype=np.float32)
    return _run(
        x,
        np.asarray(Wq, np.float32),
        np.asarray(Wk, np.float32),
        np.asarray(Wv, np.float32),
        np.asarray(Wo, np.float32),
        seq=x.shape[1],
        trace=bool(os.environ.get("BASS_TRACE")),
    )

